# revision 24
# baseline (speedup 1.0000x reference)
"""Multi-head attention (B=4, S=2048, D=512, H=8) on 8 Trainium2 NeuronCores.

Sharding: core c handles batch b = c//2 and heads [4*(c%2) .. 4*(c%2)+3]
(data parallel on B, tensor parallel on H). Each core computes Q/K/V
projections for its 4 heads, per-head attention, and a partial output
projection (its 256 rows of Wo). The host sums the two partial outputs per
batch and adds bo.

Perf design (v5): the Scalar(ACT) engine's exp stream is the hard floor
(~70us: 16 head/query-blocks x nskc chunks x 512 query-cols at 1 elem/
cycle/lane @1.2GHz). Everything else hides under it:
 - Scores for the head pair (h0: partitions 0-63, h1: 64-127) are issued
   interleaved; with K=64 each they land on disjoint PE row-groups
   (auto tile_position (0,0)/(64,0)) and run concurrently -> ~2x scores.
 - exp runs over [128, 3, 512] PSUM spans (3 key chunks, N=1536) per
   head; PSUM: 2x3 banks scores (double-buffered), 1 AV accum, 1 filler.
 - Key compaction on host: only unmasked keys ship (padded to 128), with
   an indicator row so V's ones-column (softmax denominator) falls out of
   the V projection. AV is M=65 (64 hd + denominator) per head.
 - AV for block b runs as PE filler during block b+1's exp stream (p
   persists in SBUF, bufs=2); projections/normalize/output-projection
   ride a budgeted filler queue inside the per-supergroup ACT slack.
 - Input DMAs are few and large (the ~600ns/dma_start issue cost made
   the old ramp issue-bound): K/V path split in 3 column ranges on Sync,
   Q path on Scalar, weights on Vector, late bulk on GpSimd.
 - Block order is query-block-major ((j,m) for j in 0..3 for m in 0,1)
   so output projection + out-DMA for query block j start right after
   its two head-pair blocks finish, spreading the drain.
 - All matmul operands bf16 (fp32 would halve PE rate); PSUM stays fp32.
   fp8 was tried and rejected: the error of a length-512 dot is ~5.7% of
   its own std (random-walk sum), far over the 2e-2 gate.
"""

import numpy as np
import ml_dtypes
from contextlib import ExitStack

import concourse.bass as bass
from concourse.bacc import Bacc
import concourse.mybir as mybir
import concourse.tile as tile
from concourse import bass_utils

F32 = mybir.dt.float32
BF16 = mybir.dt.bfloat16
NPBF16 = ml_dtypes.bfloat16

B, S, D, H, HD = 4, 2048, 512, 8, 64
P = 128
HPC = 4            # heads per core
NS = S // 512      # 4 query blocks of 512
VB = 65            # V' head block: 64 hd cols + the ones/indicator column


def _nblocks(total, step=512):
    return [(o, min(step, total - o)) for o in range(0, total, step)]


def _build(aug: bool, nskc: int, tk: int = 0) -> bass.Bass:
    kq = 5 if aug else 4           # x^T chunks for the Q/K projections
    sk = nskc * P                  # compacted+padded key count
    # tk>0: the last chunk holds only tk<=32 real keys; its scores for all
    # 4 heads stack into ONE [128,512] psum tile (col groups 0/32/64/96)
    # and ONE exp per query block - 1/4 of the ACT cost of a full chunk
    nm = nskc - 1 if tk else nskc  # full chunks in the main stream
    # exp supergroups of <=2 chunks (2 psum banks x 2 tiles in rotation;
    # N=1536 spans measured ~1.2ns/elem vs 1.1 at N=1024 - superlinear ACT
    # overhead - so groups of 2)
    groups = []
    c = 0
    while c < nskc:
        n = min(2, nskc - c)
        groups.append((c, n))
        c += n
    nc = Bacc(trn_type="TRN2")

    # weight wall: [P, wk | wq | wv | wo] packed partition-major on host so
    # one DMA with ~8.7KB lines loads every weight at full HBM rate
    WK0 = 0
    WQ0 = WK0 + kq * HPC * HD
    WV0 = WQ0 + kq * HPC * HD
    WALL = WV0 + 5 * HPC * VB
    ska = min(512, sk)
    xTb = nc.dram_tensor("xTb", [NS * P, kq * 512], BF16, kind="ExternalInput")
    xKa = nc.dram_tensor("xKa", [P, 5 * ska], BF16, kind="ExternalInput")
    xKb = (nc.dram_tensor("xKb", [P, 5 * (sk - ska)], BF16, kind="ExternalInput")
           if sk > ska else None)
    wall = nc.dram_tensor("wall", [P, WALL], BF16, kind="ExternalInput")
    wo = nc.dram_tensor("wo", [P, 2 * D], BF16, kind="ExternalInput")
    out = nc.dram_tensor("out", [S, D], BF16, kind="ExternalOutput")

    with tile.TileContext(nc) as tc, ExitStack() as ctx:
        sb = ctx.enter_context(tc.tile_pool(name="sb", bufs=1))
        apool = ctx.enter_context(tc.tile_pool(name="sc_ps", bufs=2, space="PSUM"))
        avpool = ctx.enter_context(tc.tile_pool(name="av_ps", bufs=2, space="PSUM"))
        fpool = ctx.enter_context(tc.tile_pool(name="f_ps", bufs=2, space="PSUM"))

        _psn = [0]

        def psum512(pro=False):
            # [128,512] fp32 psum scratch; prologue units borrow the (idle)
            # scores rotation so back-to-back units don't serialize on the
            # single filler bank
            _psn[0] += 1
            if pro:
                # prologue/drain units borrow the AV rotation (idle then) so
                # they neither chain block-0 scores behind the prologue (sc
                # rotation) nor fight the filler rotation
                return avpool.tile([P, 512], F32, tag="av", name=f"pps{_psn[0]}")
            return fpool.tile([P, 512], F32, tag="fps", name=f"ps{_psn[0]}")

        # ---------- input DMAs (few, large, long-line) ----------
        wall_t = sb.tile([P, WALL], BF16)
        xk = sb.tile([P, 5, sk], BF16)
        xt = sb.tile([P, kq, S], BF16)

        def wkt_s(t, m):
            return wall_t[:, WK0 + t * 256 + m * P: WK0 + t * 256 + (m + 1) * P]

        def wqt_s(t, m):
            return wall_t[:, WQ0 + t * 256 + m * P: WQ0 + t * 256 + (m + 1) * P]

        def wvt_s(t):
            return wall_t[:, WV0 + t * HPC * VB: WV0 + (t + 1) * HPC * VB]

        wot = sb.tile([P, 2 * D], BF16)

        def wot_s(mm):
            return wot[:, mm * D:(mm + 1) * D]

        # The DMA hardware serves all rung doorbells of a queue ROUND-ROBIN,
        # so a ramp-critical tensor sharing a queue with n-1 later ones takes
        # n x longer. FIFO-ize: a tiny DVE copy that READS the previous
        # transfer's dst and WRITES into the next one's dst region gives the
        # next dma_start a WAR dependency -> its doorbell waits.
        def gate(read_ap, write_ap):
            nc.vector.tensor_copy(write_ap, read_ap)

        nc.sync.dma_start(xk[:, :, 0:ska],
                          xKa[:, :].rearrange("p (t m) -> p t m", m=ska))
        if xKb is not None:
            nc.sync.dma_start(xk[:, :, ska:sk],
                              xKb[:, :].rearrange("p (t m) -> p t m", m=sk - ska))
        for j in range(1, NS):
            gate(xk[0:1, 0:1, sk - 8:sk], xt[0:1, 0:1, j * 512:j * 512 + 8])
            nc.sync.dma_start(
                xt[:, :, j * 512:(j + 1) * 512],
                xTb[j * P:(j + 1) * P, :].rearrange("p (t m) -> p t m", m=512))
        nc.scalar.dma_start(wall_t[:, WK0:WQ0], wall[:, WK0:WQ0])
        gate(wall_t[0:1, WK0:WK0 + 8], wall_t[0:1, WQ0:WQ0 + 8])
        nc.scalar.dma_start(wall_t[:, WQ0:WV0], wall[:, WQ0:WV0])
        gate(wall_t[0:1, WQ0:WQ0 + 8], wall_t[0:1, WV0:WV0 + 8])
        nc.scalar.dma_start(wall_t[:, WV0:], wall[:, WV0:])
        nc.gpsimd.dma_start(
            xt[:, :, 0:512],
            xTb[0:P, :].rearrange("p (t m) -> p t m", m=512))
        gate(xt[0:1, 0:1, 0:8], wot[0:1, 0:8])
        nc.gpsimd.dma_start(wot[:], wo[:, :])
        ones_t = sb.tile([P, HD], BF16, tag="ones")
        nc.vector.memset(ones_t[:], 1.0)
        # ~3.6us of junk matmuls: gets the PE HAM clock-gate to 8/8 before
        # the first real projection (otherwise the whole prologue runs at
        # 1.2GHz - the gate needs ~3.4us of sustained PE activity)
        warm_ps = fpool.tile([P, 512], F32, tag="fps", name="warm")
        for _ in range(140):
            nc.tensor.matmul(warm_ps[0:HD, 0:HD], ones_t[:], ones_t[:],
                             start=True, stop=True, skip_group_check=True)

        # ---------- projections ----------
        # per-unit tiles: a consumer then waits only on its own producer
        # unit, not every write to a monolithic K/Q/V tile
        kb = _nblocks(sk)
        ktiles = [[sb.tile([P, n], BF16, tag=f"kT{m}_{u}", name=f"kT{m}_{u}")
                   for u, (o, n) in enumerate(kb)] for m in range(2)]
        qtiles = [[sb.tile([P, 512], BF16, tag=f"qT{m}_{j}", name=f"qT{m}_{j}")
                   for j in range(NS)] for m in range(2)]
        vts = [sb.tile([P, HPC, VB], BF16, tag=f"vt{si}", name=f"vt{si}")
               for si in range(nskc)]

        def proj_kq(ws, dstt, m, src, o, n, pro=False):
            ps = psum512(pro)
            for t in range(kq):
                nc.tensor.matmul(
                    ps[:, :n], ws(t, m), src[:, t, o:o + n],
                    start=(t == 0), stop=(t == kq - 1),
                )
            nc.vector.tensor_copy(dstt[:, :n], ps[:, :n])

        def proj_v(si, pro=False):
            ps = psum512(pro)
            for t in range(5):
                nc.tensor.matmul(
                    ps[:, :HPC * VB], xk[:, t, si * P:(si + 1) * P], wvt_s(t),
                    start=(t == 0), stop=(t == 4),
                )
            nc.vector.tensor_copy(
                vts[si][:],
                ps[:, :HPC * VB].rearrange("p (l e) -> p l e", e=VB))

        # ---------- attention ----------
        vtt = sb.tile([P, VB], BF16, tag="vtt", name="vtt") if tk else None
        ptts = {}

        def vtt_prep():
            # stack the 4 heads' tail-V rows at partitions 32l (l=2m+h);
            # l>0 need a partition shift -> SBUF->SBUF DMA on gpsimd
            nc.vector.tensor_copy(vtt[0:tk, :], vts[nm][0:tk, 0, :])
            for l in range(1, HPC):
                nc.gpsimd.dma_start(vtt[32 * l:32 * l + tk, :],
                                    vts[nm][0:tk, l, :])

        opair = [sb.tile([P, S], BF16, tag=f"op{m}", name=f"op{m}") for m in range(2)]
        filler_q = []                  # (est_ns, closure) PE work units
        heavy_q = []                   # norm/outproj units (DVE-coupled)
        avq = []                       # previous blocks' AV/tail steps

        def normalize(m, j, re, ro):
            # 1/den broadcast to [128,512] by two K=1 outer products (even
            # head -> partitions 0..63, odd -> 64..127), then one multiply.
            rb = psum512()
            nc.tensor.matmul(rb[0:HD, :], ones_t[HD:HD + 1, :],
                             re[HD:HD + 1, :], start=True, stop=True)
            nc.tensor.matmul(rb[HD:P, :], ones_t[HD:HD + 1, :],
                             ro[HD:HD + 1, :], start=True, stop=True,
                             skip_group_check=True)
            sl = slice(j * 512, (j + 1) * 512)
            nc.vector.tensor_tensor(opair[m][:, sl], opair[m][:, sl], rb[:],
                                    mybir.AluOpType.mult)

        drain_mode = [False]

        def outproj_si(si):
            ps = psum512(drain_mode[0])
            for mm in range(2):
                nc.tensor.matmul(
                    ps[:], opair[mm][:, si * P:(si + 1) * P], wot_s(mm),
                    start=(mm == 0), stop=(mm == 1),
                )
            osb = sb.tile([P, D], BF16, tag="osb", bufs=3)
            nc.vector.tensor_copy(osb[:], ps[:])
            q = nc.scalar if drain_mode[0] else nc.sync
            q.dma_start(out[si * P:(si + 1) * P, :], osb[:])

        def pump(budget_ns):
            spent = 0
            for _ in range(2):
                if heavy_q:
                    est, fn = heavy_q.pop(0)
                    fn()
                    spent += est
            while avq and spent < budget_ns:
                est, fn = avq.pop(0)
                fn()
                spent += est
            while filler_q and spent < budget_ns:
                est, key, fn = filler_q.pop(0)
                fn()
                spent += est

        def ensure(keys):
            # force-emit still-queued filler units this block depends on
            # (dep semaphores are PE counters: the units must be EMITTED
            # before the consuming matmuls, or they read garbage)
            i = 0
            while i < len(filler_q):
                if filler_q[i][1] in keys:
                    filler_q.pop(i)[2]()
                else:
                    i += 1

        rrs = {}

        def make_av_steps(m, j, ptile):
            # AV + tail for block (j, m): emitted as filler during the NEXT
            # block's exp stream. Heads serialized through the single av bank.
            avs = {}

            def get_av(h):
                if h not in avs:
                    avs[h] = avpool.tile([VB, 512], F32, tag="av",
                                         name=f"av_{m}_{j}_{h}")
                return avs[h]

            def av_chunk(h, ck):
                l = 2 * m + h
                nc.tensor.matmul(get_av(h)[:], vts[ck][:, l, :],
                                 ptile[:, ck, h, :],
                                 start=(ck == 0),
                                 stop=(ck == nm - 1 and not tk))

            def av_tail(h):
                l = 2 * m + h
                nc.tensor.matmul(get_av(h)[:], vtt[32 * l:32 * l + tk, :],
                                 ptts[j][32 * l:32 * l + tk, :],
                                 start=False, stop=True,
                                 tile_position=(32 * l, 0),
                                 skip_group_check=True)

            def tail(h):
                av = get_av(h)
                # O rows into the pair tile (odd head partition-shifted
                # 0..63 -> 64..127 by an SBUF->SBUF DMA) + 1/denominator
                if h == 0:
                    nc.vector.tensor_copy(
                        opair[m][0:HD, j * 512:(j + 1) * 512], av[0:HD, :])
                else:
                    osh = sb.tile([HD, 512], BF16, tag="osh", bufs=2)
                    nc.vector.tensor_copy(osh[:], av[0:HD, :])
                    nc.gpsimd.dma_start(
                        opair[m][HD:P, j * 512:(j + 1) * 512], osh[:])
                rf = sb.tile([VB, 512], F32, tag=f"rf{h}", bufs=2,
                             name=f"rf{h}_{m}_{j}")
                # base partition must be 0 for the custom DVE op; rows
                # 0..63 produce unused garbage reciprocals of O values
                nc.vector.reciprocal_approx_fast(rf[0:VB, :], av[0:VB, :])
                rr = sb.tile([VB, 512], BF16, tag=f"rr{h}", bufs=4,
                             name=f"rr{h}_{m}_{j}")
                nc.vector.tensor_copy(rr[HD:VB, :], rf[HD:VB, :])
                rrs[h] = rr
                if h == 1:
                    re, ro = rrs[0], rrs[1]
                    heavy_q.append((500, lambda: normalize(m, j, re, ro)))
                    if m == 1:
                        for si in range(4 * j, 4 * j + 4):
                            heavy_q.append(
                                (600, (lambda s: lambda: outproj_si(s))(si)))

            steps = []
            for ck in range(nm):
                for h in range(2):
                    steps.append((213, (lambda h=h, ck=ck: av_chunk(h, ck))))
            if tk:
                for h in range(2):
                    steps.append((213, (lambda h=h: av_tail(h))))
            for h in range(2):
                steps.append((80, (lambda h=h: tail(h))))
            return steps

        prev = [None]                  # (m, j, ptile) of the previous block

        def attn_block(m, j, last=False):
            need = {("q", m, j), ("k", m)}
            if m == 1 and tk:
                need |= {("k", 0), ("q", 0, j)}
            ensure(need)
            jsl = slice(j * 512, (j + 1) * 512)
            ptile = sb.tile([P, nm, 2, 512], BF16, tag="p", bufs=2,
                            name=f"p_{m}_{j}")
            if prev[0] is not None:
                avq.extend(make_av_steps(*prev[0]))
            own = make_av_steps(m, j, ptile) if last else None
            for ck in range(nm):
                # chunk-pair tile: h0 (rows 0-63) and h1 (rows 64-127) run
                # concurrently on disjoint PE row groups; ONE exp consumes
                # both, so the list-scheduler keeps the pair adjacent
                # (separate per-head exps made it regroup by head, which
                # serialized the pair)
                sch = apool.tile([P, 2, 512], F32, tag="sc",
                                 name=f"sc_{m}_{j}_{ck}")
                u, off = (ck * P) // 512, (ck * P) % 512
                for h in range(2):
                    base = h * HD
                    nc.tensor.matmul(
                        sch[:, h, :],
                        ktiles[m][u][base:base + HD, off:off + P],
                        qtiles[m][j][base:base + HD, :],
                        start=True, stop=True,
                        tile_position=(base, 0),
                    )
                nc.scalar.activation(ptile[:, ck, :, :], sch[:, :, :],
                                     mybir.ActivationFunctionType.Exp,
                                     scale=0.125)
                if ck == 0 and m == 1 and tk:
                    # stacked tail scores: 4 concurrent (row,col)-tiled MMs,
                    # one exp for all 4 heads of this query block
                    ko = nm * P
                    u, off = ko // 512, ko % 512
                    sct = fpool.tile([P, 512], F32, tag="fps",
                                     name=f"sct_{j}")
                    for l in range(HPC):
                        ml, hl = l // 2, l % 2
                        nc.tensor.matmul(
                            sct[32 * l:32 * l + tk, :],
                            ktiles[ml][u][hl * HD:(hl + 1) * HD, off:off + tk],
                            qtiles[ml][j][hl * HD:(hl + 1) * HD, :],
                            start=True, stop=True,
                            tile_position=(hl * HD, 32 * l),
                            skip_group_check=True,
                        )
                    ptt = sb.tile([P, 512], BF16, tag="ptt", bufs=2,
                                  name=f"ptt_{j}")
                    nc.scalar.activation(ptt[:], sct[:],
                                         mybir.ActivationFunctionType.Exp,
                                         scale=0.125)
                    ptts[j] = ptt
                pump(620 if not last else 2200)
                if own is not None:
                    avq.append(own.pop(0))
                    avq.append(own.pop(0))
                    if ck == nm - 1:
                        avq.extend(own)
                        own = None
            prev[0] = None if last else (m, j, ptile)

        # ---------- emission schedule ----------
        qb = _nblocks(S)
        # prologue: ONLY what the first exps need - the dependency
        # semaphores are PE-completion counters, so every PE matmul emitted
        # before an exp transitively gates that exp. K units 1-2 (xKb-fed)
        # ride the filler queue so they land just before chunk 4 needs them.
        proj_kq(wkt_s, ktiles[0][0], 0, xk, kb[0][0], kb[0][1], pro=True)
        proj_kq(wqt_s, qtiles[0][0], 0, xt, qb[0][0], qb[0][1], pro=True)

        # fillers, in need order: V (AV of block 0 runs during block 1),
        # K(m0) u1-2 (block 0 ck4+), K(m1) + Q(m1,j0) (block 1), Q j1..j3
        filler_q += [(280, ("v",), (lambda si=si: proj_v(si)))
                     for si in range(nskc)]
        if tk:
            filler_q.append((60, ("vtt",), vtt_prep))
        filler_q += [(850, ("k", 0), (lambda u=u, o=o, n=n:
                            proj_kq(wkt_s, ktiles[0][u], 0, xk, o, n)))
                     for u, (o, n) in enumerate(kb) if u > 0]
        filler_q += [(850, ("k", 1), (lambda u=u, o=o, n=n:
                            proj_kq(wkt_s, ktiles[1][u], 1, xk, o, n)))
                     for u, (o, n) in enumerate(kb)]
        filler_q.append((850, ("q", 1, 0),
                         lambda: proj_kq(wqt_s, qtiles[1][0], 1, xt, *qb[0])))
        for jj in range(1, NS):
            for mm in range(2):
                filler_q.append(
                    (850, ("q", mm, jj),
                     (lambda mm=mm, jj=jj, o=qb[jj][0], n=qb[jj][1]:
                      proj_kq(wqt_s, qtiles[mm][jj], mm, xt, o, n))))

        for j in range(NS):
            for m in range(2):
                attn_block(m, j, last=(j == NS - 1 and m == 1))

        # drain leftovers (outproj switches to the idle AV psum rotation
        # and the idle Scalar DMA queue); junk MMs between units keep the
        # PE clock warm through the dependency chains
        drain_mode[0] = True
        djunk = apool.tile([P, 2, 512], F32, tag="sc", name="djunk")

        def dwarm(n):
            for _ in range(n):
                nc.tensor.matmul(djunk[0:HD, 0, 0:HD], ones_t[:], ones_t[:],
                                 start=True, stop=True, skip_group_check=True)

        while avq:
            avq.pop(0)[1]()
            dwarm(1)
        while heavy_q:
            heavy_q.pop(0)[1]()
            dwarm(3)
        while filler_q:
            filler_q.pop(0)[2]()
            dwarm(1)

    nc.compile()
    return nc


def kernel(x, mask, Wq, bq, Wk, bk, Wv, bv, Wo, bo):
    x = np.asarray(x, np.float32)
    mask = np.asarray(mask)
    Wq, bq = np.asarray(Wq, np.float32), np.asarray(bq, np.float32)
    Wk, bk = np.asarray(Wk, np.float32), np.asarray(bk, np.float32)
    Wv, bv = np.asarray(Wv, np.float32), np.asarray(bv, np.float32)
    Wo, bo = np.asarray(Wo, np.float32), np.asarray(bo, np.float32)

    aug = any(np.any(bias != 0) for bias in (bq, bk, bv))
    kq = 5 if aug else 4

    counts = mask.sum(axis=1)
    mc = max(int(c) for c in counts)
    sk = max(P, int(-(-mc // P) * P))
    sk = min(sk, S)
    nskc = sk // P
    tk = mc - (nskc - 1) * P           # real keys in the last chunk
    if not (0 < tk <= 32 and nskc >= 2):
        tk = 0                         # fall back to the uniform path

    in_maps = []
    for c in range(8):
        b, half = c // 2, c % 2
        hs = slice(half * HPC * HD, (half + 1) * HPC * HD)   # 256 head columns

        idx = np.nonzero(mask[b])[0]
        su = len(idx)

        xT = np.zeros((kq * P, S), np.float32)
        xT[:D] = x[b].T
        xKT = np.zeros((5 * P, sk), np.float32)
        xKT[:D, :su] = x[b].T[:, idx]
        xKT[D, :su] = 1.0                      # real-key indicator row
        if aug:
            xT[D] = 1.0
        # j-blocked, partition-major: one 4KB-line DMA per query block
        xTb = (xT.reshape(kq, P, S // 512, 512).transpose(2, 1, 0, 3)
               .reshape((S // 512) * P, kq * 512))

        wq_a = np.zeros((kq * P, HPC * HD), np.float32)
        wq_a[:D] = Wq[:, hs]
        wk_a = np.zeros((kq * P, HPC * HD), np.float32)
        wk_a[:D] = Wk[:, hs]

        wv_a = np.zeros((5 * P, HPC * VB), np.float32)
        for l in range(HPC):
            hg = half * HPC + l
            wv_a[:D, l * VB:l * VB + HD] = Wv[:, hg * HD:(hg + 1) * HD]
            wv_a[D, l * VB + HD] = 1.0         # indicator -> ones column
        if aug:
            wq_a[D] = bq[hs]
            wk_a[D] = bk[hs]
            for l in range(HPC):
                hg = half * HPC + l
                wv_a[D, l * VB:l * VB + HD] = bv[hg * HD:(hg + 1) * HD]

        wo_a = np.stack(
            [Wo[(half * HPC + 2 * m) * HD:(half * HPC + 2 * m + 2) * HD, :]
             for m in range(2)]
        ).astype(np.float32)

        # pack the pre-attention weights partition-major into one wall;
        # wo ships separately (needed late) to keep the wall transfer short
        wall = np.concatenate([
            wk_a.reshape(kq, P, HPC * HD).transpose(1, 0, 2).reshape(P, -1),
            wq_a.reshape(kq, P, HPC * HD).transpose(1, 0, 2).reshape(P, -1),
            wv_a.reshape(5, P, HPC * VB).transpose(1, 0, 2).reshape(P, -1),
        ], axis=1)
        wo_p = wo_a.transpose(1, 0, 2).reshape(P, -1)

        ska = min(512, sk)
        xk5 = xKT.reshape(5, P, sk)
        im = {
            "xTb": xTb.astype(NPBF16),
            "xKa": xk5[:, :, :ska].transpose(1, 0, 2).reshape(P, -1).astype(NPBF16),
            "wall": wall.astype(NPBF16),
            "wo": wo_p.astype(NPBF16),
        }
        if sk > ska:
            im["xKb"] = (xk5[:, :, ska:].transpose(1, 0, 2)
                         .reshape(P, -1).astype(NPBF16))
        in_maps.append(im)

    nc = _build(aug, nskc, tk)
    import os
    trace = bool(int(os.environ.get("MHA_TRACE", "0")))
    res = bass_utils.run_bass_kernel_spmd(nc, in_maps, core_ids=list(range(8)),
                                          trace=trace)
    global last_result
    last_result = res

    outf = np.empty((B, S, D), np.float32)
    for b in range(B):
        outf[b] = (res.results[2 * b]["out"].astype(np.float32)
                   + res.results[2 * b + 1]["out"].astype(np.float32)
                   + bo[None, :])
    return outf
